# revision 1
# baseline (speedup 1.0000x reference)
"""Trainium2 Bass kernel for nn_DroneRelationModel (8 NeuronCores).

Strategy:
  - Attention sharded (head, query-half) across 8 cores; scores computed
    transposed in PSUM (row-packed K=32 matmuls), ACT exp, AV matmul with a
    ones-column producing softmax denominators, normalize.
  - AllGather context pieces; every core folds out_proj+w1 into per-node
    tables A,B (|w2|-scaled, sign-permuted hidden, biases folded in).
  - Pair head sharded by pair slices. Pairs are bucketed by src block
    (32 blocks of 128 nodes, each bucket padded to a uniform width BI):
    the src-side rows come from one-hot matmuls against the A table in
    SBUF (one matmul per 128-pair tile, host-uploaded one-hot), the
    dst-side rows via dma_gather from the B table in DRAM. The dst
    gathers are issued as prepare_only so Q7 descriptor generation
    overlaps the attention phase; triggers fire once the B table lands.
    Then add + relu (tensor_tensor max) + sign-split segmented reduce
    gives the w2-dot, sigmoid via tanh.
All heavy matmul inputs in bf16 (validated ~0.2% max rel err end to end).
"""
import numpy as np
import ml_dtypes

N, H, HEADS, P = 4096, 128, 4, 262144
DH = 32
NCORES = 8
NQ = 2048
QB = 1024
PC = P // NCORES
GCHUNK = 4096
RREP = 3


def _build(npos, b2val, BI):
    import concourse.bass as bass
    import concourse.mybir as mybir
    import concourse.tile as tile
    from concourse import bacc

    F32 = mybir.dt.float32
    BF16 = mybir.dt.bfloat16
    I16 = mybir.dt.int16
    AF = mybir.ActivationFunctionType
    ALU = mybir.AluOpType

    C = 32 * BI               # padded pair-column count
    NGC = C // GCHUNK         # gather chunks
    TPB = BI // 128           # 128-pair tiles per src block

    nc = bacc.Bacc("TRN2", target_bir_lowering=False, debug=False,
                   num_devices=NCORES, dynamic_dma_scratch_size=32768)

    xT = nc.dram_tensor("xT", [H, N], BF16, kind="ExternalInput")
    xTq = nc.dram_tensor("xTq", [H, NQ], BF16, kind="ExternalInput")
    wq_t = nc.dram_tensor("wq_t", [H, DH * RREP], BF16, kind="ExternalInput")
    wk_t = nc.dram_tensor("wk_t", [H, DH * RREP], BF16, kind="ExternalInput")
    wv_t = nc.dram_tensor("wv_t", [H, DH], BF16, kind="ExternalInput")
    bq = nc.dram_tensor("bq", [DH * RREP, 1], F32, kind="ExternalInput")
    wa_t = nc.dram_tensor("wa_t", [H, H], BF16, kind="ExternalInput")
    wb_t = nc.dram_tensor("wb_t", [H, H], BF16, kind="ExternalInput")
    bias_r = nc.dram_tensor("bias_r", [1, H], BF16, kind="ExternalInput")
    dstw = nc.dram_tensor("dstw", [128, C // 16], I16, kind="ExternalInput")
    oneh = nc.dram_tensor("oneh", [128, C], BF16, kind="ExternalInput")
    preds = nc.dram_tensor("preds", [128, C // 128], F32, kind="ExternalOutput")
    cc_in = nc.dram_tensor("cc_in", [DH, NQ], BF16)
    cc_out = nc.dram_tensor("cc_out", [NCORES, DH, NQ], BF16, addr_space="Shared")
    btab_d = nc.dram_tensor("btab_d", [N, H], BF16)

    with tile.TileContext(nc) as tc:
        with tc.tile_pool(name="const", bufs=1) as cpool:
            def cload(name, dram, shape, dtype):
                t = cpool.tile(shape, dtype, tag=name)
                nc.sync.dma_start(out=t[...], in_=dram[...])
                return t

            xT_sb = cload("xT", xT, [H, N], BF16)
            xTq_sb = cload("xTq", xTq, [H, NQ], BF16)
            wq_sb = cload("wq", wq_t, [H, DH * RREP], BF16)
            wk_sb = cload("wk", wk_t, [H, DH * RREP], BF16)
            wv_sb = cload("wv", wv_t, [H, DH], BF16)
            bq_sb = cload("bq", bq, [DH * RREP, 1], F32)
            wa_sb = cload("wa", wa_t, [H, H], BF16)
            wb_sb = cload("wb", wb_t, [H, H], BF16)
            bias_sb = cload("bias", bias_r, [1, H], BF16)
            dst_sb = cload("dst", dstw, [128, C // 16], I16)

            # dst-gather output tiles (addresses fixed before preps run)
            dg_tiles = []
            for g in range(NGC):
                dg = cpool.tile([128, GCHUNK // 128, 128], BF16, tag=f"dg{g}")
                dg_tiles.append(dg)

            kT_sb = cpool.tile([DH * RREP, N], BF16, tag="kT")
            qT_sb = cpool.tile([DH * RREP, NQ], BF16, tag="qT")
            v_sb = cpool.tile([128, 33 * 32], BF16, tag="v")
            ctxn_sb = cpool.tile([DH, NQ], BF16, tag="ctxn")
            ones_sb = cpool.tile([1, H], BF16, tag="ones")
            nc.vector.memset(ones_sb[...], 1.0)
            zeros_sb = cpool.tile([128, 512], BF16, tag="zeros")
            nc.vector.memset(zeros_sb[...], 0.0)

            with (
                tc.tile_pool(name="kqv_ps", bufs=2, space="PSUM") as kqv_ps,
                tc.tile_pool(name="v_ps", bufs=4, space="PSUM") as v_ps,
            ):
                for i in range(4):
                    ps = kqv_ps.tile([DH * RREP, QB], F32, tag="kq")
                    for j in range(2):
                        nc.tensor.matmul(ps[:, j * 512:(j + 1) * 512], wk_sb[...],
                                         xT_sb[:, i * QB + j * 512:i * QB + (j + 1) * 512],
                                         start=True, stop=True)
                    nc.vector.tensor_copy(kT_sb[:, i * QB:(i + 1) * QB], ps[...])
                for i in range(2):
                    ps = kqv_ps.tile([DH * RREP, QB], F32, tag="kq")
                    for j in range(2):
                        nc.tensor.matmul(ps[:, j * 512:(j + 1) * 512], wq_sb[...],
                                         xTq_sb[:, i * QB + j * 512:i * QB + (j + 1) * 512],
                                         start=True, stop=True)
                    nc.vector.tensor_scalar_add(qT_sb[:, i * QB:(i + 1) * QB],
                                                ps[...], bq_sb[...])
                nc.vector.memset(v_sb[...], 1.0)
                for kc in range(32):
                    ps = v_ps.tile([128, DH], F32, tag="v")
                    nc.tensor.matmul(ps[...], xT_sb[:, kc * 128:(kc + 1) * 128],
                                     wv_sb[...], start=True, stop=True)
                    nc.vector.tensor_copy(v_sb[:, kc * 33:kc * 33 + 32], ps[...])

            with (
                tc.tile_pool(name="s_ps", bufs=3, space="PSUM") as s_ps,
                tc.tile_pool(name="av_ps", bufs=1, space="PSUM") as av_psp,
                tc.tile_pool(name="es", bufs=6) as es_pool,
                tc.tile_pool(name="misc", bufs=2) as misc,
            ):
                for qb in range(2):
                    av_ps = av_psp.tile([128, QB], F32, tag="av")
                    es_tiles = {}
                    for kc in range(32):
                        r = kc % RREP
                        sp = s_ps.tile([128, QB], F32, tag="s")
                        for j in range(2):
                            nc.tensor.matmul(
                                sp[:, j * 512:(j + 1) * 512],
                                kT_sb[r * DH:(r + 1) * DH, kc * 128:(kc + 1) * 128],
                                qT_sb[r * DH:(r + 1) * DH,
                                      qb * QB + j * 512:qb * QB + (j + 1) * 512],
                                start=True, stop=True, tile_position=(r * DH, 0))
                        es = es_pool.tile([128, QB], BF16, tag="es")
                        nc.scalar.activation(es[...], sp[...], AF.Exp)
                        es_tiles[kc] = es
                        if kc % 2 == 1:
                            for t, kk in ((0, kc - 1), (1, kc)):
                                for j in range(2):
                                    nc.tensor.matmul(
                                        av_ps[t * 64:t * 64 + 33, j * 512:(j + 1) * 512],
                                        v_sb[:, kk * 33:(kk + 1) * 33],
                                        es_tiles[kk][:, j * 512:(j + 1) * 512],
                                        start=(kk < 2), stop=(kk >= 30),
                                        tile_position=(0, t * 64))
                            es_tiles.clear()
                    craw = misc.tile([33, QB], F32, tag="craw")
                    t1c = misc.tile([33, QB], F32, tag="t1c")
                    nc.vector.tensor_copy(t1c[...], av_ps[64:97, :])
                    nc.vector.tensor_add(craw[...], av_ps[0:33, :], t1c[...])
                    r_sb = misc.tile([1, QB], F32, tag="r")
                    nc.vector.reciprocal(r_sb[...], craw[32:33, :])
                    rbf = misc.tile([1, QB], BF16, tag="rbf")
                    nc.vector.tensor_copy(rbf[...], r_sb[...])
                    bc_ps = s_ps.tile([DH, QB], F32, tag="s")
                    for j in range(2):
                        nc.tensor.matmul(bc_ps[:, j * 512:(j + 1) * 512],
                                         ones_sb[:, 0:DH], rbf[:, j * 512:(j + 1) * 512],
                                         start=True, stop=True)
                    nc.vector.tensor_mul(ctxn_sb[:, qb * QB:(qb + 1) * QB],
                                         craw[0:32, :], bc_ps[...])

            nc.sync.dma_start(out=cc_in[...], in_=ctxn_sb[...])
            nc.gpsimd.collective_compute(
                "AllGather", ALU.bypass, replica_groups=[list(range(NCORES))],
                ins=[cc_in.ap()], outs=[cc_out.ap()])
            ctxT_sb = cpool.tile([H, N], BF16, tag="ctxT")
            for g in range(NCORES):
                hh, half = g // 2, g % 2
                nc.sync.dma_start(
                    out=ctxT_sb[hh * DH:(hh + 1) * DH, half * NQ:(half + 1) * NQ],
                    in_=cc_out[g, :, :])

            atab = cpool.tile([128, 32 * 128], BF16, tag="atab")
            btab = cpool.tile([128, 32 * 128], BF16, tag="btab")
            with tc.tile_pool(name="tab_ps", bufs=2, space="PSUM") as tab_ps:
                for t in range(32):
                    pa = tab_ps.tile([128, 128], F32, tag="ta")
                    pb = tab_ps.tile([128, 128], F32, tag="tb")
                    nc.tensor.matmul(pa[...], ones_sb[...], bias_sb[...],
                                     start=True, stop=False)
                    nc.tensor.matmul(pa[...], ctxT_sb[:, t * 128:(t + 1) * 128],
                                     wa_sb[...], start=False, stop=True)
                    nc.tensor.matmul(pb[...], ctxT_sb[:, t * 128:(t + 1) * 128],
                                     wb_sb[...], start=True, stop=True)
                    nc.vector.tensor_copy(atab[:, t * 128:(t + 1) * 128], pa[...])
                    nc.vector.tensor_copy(btab[:, t * 128:(t + 1) * 128], pb[...])
            nc.sync.dma_start(out=btab_d.ap().rearrange("(c p) d -> p c d", p=128),
                              in_=btab[...].rearrange("p (c d) -> p c d", d=128))

            # dst gathers: prep (fast desc-gen) + trigger per chunk; drains
            # of chunk g overlap desc-gen of g+1 and the head compute below
            dg_sems = []
            for g in range(NGC):
                sem = nc.alloc_semaphore(f"dgsem{g}")
                dg_sems.append(sem)
                iw = GCHUNK // 16
                nc.gpsimd.dma_gather(
                    dg_tiles[g][...], btab_d[...],
                    dst_sb[:, g * iw:(g + 1) * iw],
                    num_idxs=GCHUNK, num_idxs_reg=GCHUNK, elem_size=128,
                    transpose=False, single_packet=False,
                    prepare_only=True, sem=sem)
                nc.gpsimd.trigger_dma(count=None)

            with (
                tc.tile_pool(name="oh", bufs=3) as oh_pool,
                tc.tile_pool(name="src_ps", bufs=8, space="PSUM") as src_ps,
                tc.tile_pool(name="hseq", bufs=3) as hpool,
                tc.tile_pool(name="sig", bufs=1) as sig_pool,
            ):
                NB = GCHUNK // 128
                lsp = sig_pool.tile([128, C // 128], F32, tag="lsp")
                lsn = sig_pool.tile([128, C // 128], F32, tag="lsn")
                for g in range(NGC):
                    oh_sb = oh_pool.tile([128, GCHUNK], BF16, tag="oh")
                    nc.sync.dma_start(out=oh_sb[...],
                                      in_=oneh[:, g * GCHUNK:(g + 1) * GCHUNK])
                    dg = dg_tiles[g]
                    for b in range(NB // 4):
                        ps = src_ps.tile([128, 512], F32, tag="sp")
                        for t in range(4):
                            gt = g * NB + b * 4 + t      # global 128-pair tile
                            I = gt // TPB                # src block index
                            nc.tensor.matmul(
                                ps[:, t * 128:(t + 1) * 128],
                                oh_sb[:, (b * 4 + t) * 128:(b * 4 + t + 1) * 128],
                                atab[:, I * 128:(I + 1) * 128],
                                start=True, stop=True)
                        hs = hpool.tile([128, 512], BF16, tag="hs")
                        nc.vector.tensor_add(
                            hs[...], ps[...],
                            dg[:, b * 4:(b + 1) * 4, :].rearrange("p b d -> p (b d)")
                        )._wait_ge(dg_sems[g], 16)
                        hr = hpool.tile([128, 4, 128], BF16, tag="hr")
                        nc.vector.tensor_max(
                            hr[...].rearrange("p b d -> p (b d)"), hs[...],
                            zeros_sb[...])
                        ob = g * NB + b * 4
                        nc.vector.tensor_reduce(
                            lsp[:, ob:ob + 4].rearrange("p b -> p b ()"),
                            hr[:, :, 0:npos], op=ALU.add, axis=mybir.AxisListType.X)
                        nc.vector.tensor_reduce(
                            lsn[:, ob:ob + 4].rearrange("p b -> p b ()"),
                            hr[:, :, npos:H], op=ALU.add, axis=mybir.AxisListType.X)
                logit = sig_pool.tile([128, C // 128], F32, tag="logit")
                nc.vector.tensor_sub(logit[...], lsp[...], lsn[...])
                sig_sb = sig_pool.tile([128, C // 128], F32, tag="sig")
                nc.scalar.activation(sig_sb[...], logit[...], AF.Tanh,
                                     scale=0.5, bias=0.5 * b2val)
                nc.vector.tensor_scalar(sig_sb[...], sig_sb[...], 0.5, 0.5,
                                        op0=ALU.mult, op1=ALU.add)
                nc.sync.dma_start(out=preds[...], in_=sig_sb[...])
    nc.compile()
    return nc


def _prep_inputs(inputs):
    x = np.asarray(inputs["context_embeddings"], np.float32)
    ipw = np.asarray(inputs["in_proj_w"], np.float32)
    ipb = np.asarray(inputs["in_proj_b"], np.float32)
    opw = np.asarray(inputs["out_proj_w"], np.float32)
    opb = np.asarray(inputs["out_proj_b"], np.float32)
    w1 = np.asarray(inputs["w1"], np.float32)
    b1 = np.asarray(inputs["b1"], np.float32)
    w2v = np.asarray(inputs["w2"], np.float32)[0]
    rel = np.asarray(inputs["relationships"])

    bf = ml_dtypes.bfloat16
    scale = 1.0 / np.sqrt(DH)
    xT_a = np.ascontiguousarray(x.T).astype(bf)
    w1a, w1b = w1[:, :H], w1[:, H:]
    WA = w1a @ opw
    WB = w1b @ opw
    bv_full = ipb[2 * H:]
    node_bias = opw @ bv_full + opb
    bias_total = ((w1a + w1b) @ node_bias + b1).astype(np.float32)
    perm = np.argsort(w2v < 0, kind="stable")
    npos = int((w2v >= 0).sum())
    aw2 = np.abs(w2v)[perm]
    WA_t = np.ascontiguousarray((WA[perm] * aw2[:, None]).T).astype(bf)
    WB_t = np.ascontiguousarray((WB[perm] * aw2[:, None]).T).astype(bf)
    bias_perm = (bias_total[perm] * aw2).astype(np.float32)

    # --- pair bucketing by src block ---
    src_all = rel[:, 0].astype(np.int64).reshape(NCORES, PC)
    dst_all = rel[:, 1].astype(np.int64).reshape(NCORES, PC)
    blk = src_all // 128
    maxb = 0
    for c in range(NCORES):
        maxb = max(maxb, int(np.bincount(blk[c], minlength=32).max()))
    BI = ((maxb + 127) // 128) * 128
    C = 32 * BI

    def wrap_idx(v, ngc):
        a = v.astype(np.int16).reshape(ngc, GCHUNK // 16, 16).transpose(0, 2, 1)
        b = a.transpose(1, 0, 2).reshape(16, ngc * GCHUNK // 16)
        return np.ascontiguousarray(np.tile(b, (8, 1)))

    in_maps = []
    col_of_pair = np.empty((NCORES, PC), np.int64)
    for c in range(NCORES):
        h, half = c // 2, c % 2
        qoff = half * NQ
        src, dst = src_all[c], dst_all[c]
        order = np.argsort(blk[c], kind="stable")
        counts = np.bincount(blk[c], minlength=32)
        starts = np.concatenate([[0], np.cumsum(counts)[:-1]])
        # column index for sorted position i in bucket I: I*BI + (i - starts[I])
        bs = blk[c][order]
        cols_sorted = bs * BI + (np.arange(PC) - starts[bs])
        col_of_pair[c][order] = cols_sorted

        dst_cols = np.zeros(C, np.int64)
        dst_cols[cols_sorted] = dst[order]
        onehot = np.zeros((128, C), np.float32)
        onehot[src[order] % 128, cols_sorted] = 1.0

        wqh = (ipw[DH * h:DH * (h + 1), :] * scale).T
        wkh = ipw[H + DH * h:H + DH * (h + 1), :].T
        wvh = ipw[2 * H + DH * h:2 * H + DH * (h + 1), :].T
        bqh = ipb[DH * h:DH * (h + 1)] * scale
        in_maps.append({
            "xT": xT_a,
            "xTq": np.ascontiguousarray(xT_a[:, qoff:qoff + NQ]),
            "wq_t": np.ascontiguousarray(np.tile(wqh, (1, RREP))).astype(bf),
            "wk_t": np.ascontiguousarray(np.tile(wkh, (1, RREP))).astype(bf),
            "wv_t": np.ascontiguousarray(wvh).astype(bf),
            "bq": np.tile(bqh, RREP).reshape(DH * RREP, 1).astype(np.float32),
            "wa_t": WA_t, "wb_t": WB_t,
            "bias_r": bias_perm.reshape(1, H).astype(bf),
            "dstw": wrap_idx(dst_cols, C // GCHUNK),
            "oneh": np.ascontiguousarray(onehot.astype(bf)),
        })
    return in_maps, npos, BI, col_of_pair


def kernel(**inputs):
    from concourse import bass_utils

    in_maps, npos, BI, col_of_pair = _prep_inputs(inputs)
    b2val = float(np.asarray(inputs["b2"], np.float32)[0])
    nc = _build(npos, b2val, BI)
    res = bass_utils.run_bass_kernel_spmd(
        nc, in_maps, core_ids=list(range(NCORES)), trace=False)
    out = np.empty(P, np.float32)
    for c in range(NCORES):
        pr = res.results[c]["preds"]           # [128, C//128]
        flat = pr.T.reshape(-1)                # col-major: col = b*128 + p
        out[c * PC:(c + 1) * PC] = flat[col_of_pair[c]]
    return out.astype(np.float32)



# revision 10
# speedup vs baseline: 1.5560x; 1.5560x over previous
"""Trainium2 Bass kernel for nn_DroneRelationModel (8 NeuronCores).

Strategy v3 (gather-overlap):
  - Attention sharded (head, query-half) across 8 cores; the AllGather is
    split per query-quarter (qb) so the B-table halves land earlier.
  - dst-gather desc-gen (the bottleneck: ~7-8ns/idx on GpSimd) runs from
    t~0: prepare_only dma_gathers are emitted BEFORE any write to btab_d
    (so Tile attaches no RAW dep to them), interleaved with the two
    collective_computes on the gpsimd queue.  All ~35k descriptors can sit
    untriggered because they are spread over TWO SWDGE queues with a 64KB
    descriptor ring carveout (measured: one 32KB queue caps out between
    18k and 23k pending indices and the kernel deadlocks).
  - Trigger gating: a tiny dummy prep emitted AFTER the btab stores picks
    up Tile's RAW waits on the four store DMAs; the per-queue triggers
    behind it fire only once the B table has landed.  Later chunks get
    their own triggers for drain/compute pipelining.
  - Pair bucketing compacted: pairs rebalanced across cores per src-block
    (round-robin deal), so every (core, block) bucket fits in
    kb = roundup(ceil(count_b/8), 128) columns -> C ~ 34k instead of 41k.
  - Head: src rows via one-hot matmuls vs A table (SBUF), dst rows via the
    gathers, add on DVE, relu on Scalar (ACT), sign-split reduces on DVE.
All heavy matmul inputs in bf16.
"""
import numpy as np
import ml_dtypes

N, H, HEADS, P = 4096, 128, 4, 262144
DH = 32
NCORES = 8
NQ = 2048
QB = 1024
RREP = 3
NCHUNK = 8


def _build(npos, b2val, C, sizes, i_of_tile):
    import concourse.bass as bass
    import concourse.mybir as mybir
    import concourse.tile as tile
    from concourse import bacc

    F32 = mybir.dt.float32
    BF16 = mybir.dt.bfloat16
    I16 = mybir.dt.int16
    AF = mybir.ActivationFunctionType
    ALU = mybir.AluOpType

    nc = bacc.Bacc("TRN2", target_bir_lowering=False, debug=False,
                   num_devices=NCORES, dynamic_dma_scratch_size=65536,
                   num_swdge_queues=2)

    xT = nc.dram_tensor("xT", [H, N], BF16, kind="ExternalInput")
    xTq = nc.dram_tensor("xTq", [H, NQ], BF16, kind="ExternalInput")
    wq_t = nc.dram_tensor("wq_t", [H, DH * RREP], BF16, kind="ExternalInput")
    wk_t = nc.dram_tensor("wk_t", [H, DH * RREP], BF16, kind="ExternalInput")
    wv_t = nc.dram_tensor("wv_t", [H, DH], BF16, kind="ExternalInput")
    bq = nc.dram_tensor("bq", [DH * RREP, 1], F32, kind="ExternalInput")
    wa_t = nc.dram_tensor("wa_t", [H, H], BF16, kind="ExternalInput")
    wb_t = nc.dram_tensor("wb_t", [H, H], BF16, kind="ExternalInput")
    bias_r = nc.dram_tensor("bias_r", [1, H], BF16, kind="ExternalInput")
    dstw = nc.dram_tensor("dstw", [128, C // 16], I16, kind="ExternalInput")
    oneh = nc.dram_tensor("oneh", [128, C], BF16, kind="ExternalInput")
    preds = nc.dram_tensor("preds", [128, C // 128], F32, kind="ExternalOutput")
    cc_in = [nc.dram_tensor(f"cc_in{k}", [DH, QB], BF16) for k in range(2)]
    cc_out = [nc.dram_tensor(f"cc_out{k}", [NCORES, DH, QB], BF16,
                             addr_space="Shared") for k in range(2)]
    btab_d = nc.dram_tensor("btab_d", [N, H], BF16)

    offs = np.concatenate([[0], np.cumsum(sizes)]).astype(int)

    with tile.TileContext(nc) as tc:
        with tc.tile_pool(name="const", bufs=1) as cpool:
            def cload(pool, name, dram, shape, dtype):
                t = pool.tile(shape, dtype, tag=name)
                nc.sync.dma_start(out=t[...], in_=dram[...])
                return t

            # index data first so preps can start early
            dst_sb = cload(cpool, "dst", dstw, [128, C // 16], I16)
            wq_sb = cload(cpool, "wq", wq_t, [H, DH * RREP], BF16)
            wk_sb = cload(cpool, "wk", wk_t, [H, DH * RREP], BF16)
            wv_sb = cload(cpool, "wv", wv_t, [H, DH], BF16)
            bq_sb = cload(cpool, "bq", bq, [DH * RREP, 1], F32)
            wa_sb = cload(cpool, "wa", wa_t, [H, H], BF16)
            wb_sb = cload(cpool, "wb", wb_t, [H, H], BF16)
            bias_sb = cload(cpool, "bias", bias_r, [1, H], BF16)

            # dst-gather output tiles (addresses fixed before preps run)
            dg_tiles = []
            for g in range(NCHUNK):
                dg = cpool.tile([128, sizes[g] // 128, 128], BF16, tag=f"dg{g}")
                dg_tiles.append(dg)
            dgdummy = cpool.tile([128, 1, 128], BF16, tag="dgdummy")
            dgdummy1 = cpool.tile([128, 1, 128], BF16, tag="dgdummy1")
            atab = cpool.tile([128, 32 * 128], BF16, tag="atab")

            dg_sems = [nc.alloc_semaphore(f"dgsem{g}") for g in range(NCHUNK)]
            dummy_sem = nc.alloc_semaphore("dummysem")
            dummy_sem1 = nc.alloc_semaphore("dummysem1")

            def prep(g):
                iw0, iw1 = offs[g] // 16, offs[g + 1] // 16
                nc.gpsimd.dma_gather(
                    dg_tiles[g][...], btab_d[...],
                    dst_sb[:, iw0:iw1],
                    num_idxs=sizes[g], num_idxs_reg=sizes[g], elem_size=128,
                    transpose=False, single_packet=False,
                    prepare_only=True, sem=dg_sems[g], queue_num=g % 2)

            prep(0)
            prep(1)
            prep(2)

            with tc.tile_pool(name="attn", bufs=1) as apool:
                xT_sb = cload(apool, "xT", xT, [H, N], BF16)
                xTq_sb = cload(apool, "xTq", xTq, [H, NQ], BF16)
                kT_sb = apool.tile([DH * RREP, N], BF16, tag="kT")
                qT_sb = apool.tile([DH * RREP, NQ], BF16, tag="qT")
                v_sb = apool.tile([128, 33 * 32], BF16, tag="v")
                ctxn_sb = apool.tile([DH, NQ], BF16, tag="ctxn")
                ctxT_sb = apool.tile([H, N], BF16, tag="ctxT")
                btab = apool.tile([128, 32 * 128], BF16, tag="btab")
                ones_sb = apool.tile([1, H], BF16, tag="ones")
                nc.vector.memset(ones_sb[...], 1.0)

                # ---- qkv projections ----
                with tc.tile_pool(name="kqv_ps", bufs=2, space="PSUM") as kqv_ps:
                    for i in range(4):
                        ps = kqv_ps.tile([DH * RREP, QB], F32, tag="kq")
                        for j in range(2):
                            nc.tensor.matmul(ps[:, j * 512:(j + 1) * 512], wk_sb[...],
                                             xT_sb[:, i * QB + j * 512:i * QB + (j + 1) * 512],
                                             start=True, stop=True)
                        nc.vector.tensor_copy(kT_sb[:, i * QB:(i + 1) * QB], ps[...])
                    for i in range(2):
                        ps = kqv_ps.tile([DH * RREP, QB], F32, tag="kq")
                        for j in range(2):
                            nc.tensor.matmul(ps[:, j * 512:(j + 1) * 512], wq_sb[...],
                                             xTq_sb[:, i * QB + j * 512:i * QB + (j + 1) * 512],
                                             start=True, stop=True)
                        nc.vector.tensor_scalar_add(qT_sb[:, i * QB:(i + 1) * QB],
                                                    ps[...], bq_sb[...])
                    nc.vector.memset(v_sb[...], 1.0)
                    with tc.tile_pool(name="v_ps", bufs=4, space="PSUM") as v_ps:
                        for kc in range(32):
                            ps = v_ps.tile([128, DH], F32, tag="v")
                            nc.tensor.matmul(ps[...], xT_sb[:, kc * 128:(kc + 1) * 128],
                                             wv_sb[...], start=True, stop=True)
                            nc.vector.tensor_copy(v_sb[:, kc * 33:kc * 33 + 32], ps[...])

                with (
                    tc.tile_pool(name="s_ps", bufs=2, space="PSUM") as s_ps,
                    tc.tile_pool(name="av_ps", bufs=1, space="PSUM") as av_psp,
                    tc.tile_pool(name="tab_ps", bufs=2, space="PSUM") as tab_ps,
                    tc.tile_pool(name="es", bufs=4) as es_pool,
                    tc.tile_pool(name="misc", bufs=1) as misc,
                ):
                    def attention_qb(qb):
                        av_ps = av_psp.tile([128, QB], F32, tag="av")
                        es_tiles = {}
                        for kc in range(32):
                            r = kc % RREP
                            sp = s_ps.tile([128, QB], F32, tag="s")
                            for j in range(2):
                                nc.tensor.matmul(
                                    sp[:, j * 512:(j + 1) * 512],
                                    kT_sb[r * DH:(r + 1) * DH, kc * 128:(kc + 1) * 128],
                                    qT_sb[r * DH:(r + 1) * DH,
                                          qb * QB + j * 512:qb * QB + (j + 1) * 512],
                                    start=True, stop=True, tile_position=(r * DH, 0))
                            es = es_pool.tile([128, QB], BF16, tag="es")
                            nc.scalar.activation(es[...], sp[...], AF.Exp)
                            es_tiles[kc] = es
                            if kc % 2 == 1:
                                for t, kk in ((0, kc - 1), (1, kc)):
                                    for j in range(2):
                                        nc.tensor.matmul(
                                            av_ps[t * 64:t * 64 + 33, j * 512:(j + 1) * 512],
                                            v_sb[:, kk * 33:(kk + 1) * 33],
                                            es_tiles[kk][:, j * 512:(j + 1) * 512],
                                            start=(kk < 2), stop=(kk >= 30),
                                            tile_position=(0, t * 64),
                                            skip_group_check=True)
                                es_tiles.clear()
                        craw = misc.tile([33, QB], F32, tag="craw")
                        t1c = misc.tile([33, QB], F32, tag="t1c")
                        nc.vector.tensor_copy(t1c[...], av_ps[64:97, :])
                        nc.vector.tensor_add(craw[...], av_ps[0:33, :], t1c[...])
                        r_sb = misc.tile([1, QB], F32, tag="r")
                        nc.vector.reciprocal(r_sb[...], craw[32:33, :])
                        rbf = misc.tile([1, QB], BF16, tag="rbf")
                        nc.vector.tensor_copy(rbf[...], r_sb[...])
                        bc_ps = s_ps.tile([DH, QB], F32, tag="s")
                        for j in range(2):
                            nc.tensor.matmul(bc_ps[:, j * 512:(j + 1) * 512],
                                             ones_sb[:, 0:DH], rbf[:, j * 512:(j + 1) * 512],
                                             start=True, stop=True)
                        nc.vector.tensor_mul(ctxn_sb[:, qb * QB:(qb + 1) * QB],
                                             craw[0:32, :], bc_ps[...])
                        nc.sync.dma_start(out=cc_in[qb][...],
                                          in_=ctxn_sb[:, qb * QB:(qb + 1) * QB])

                    def assemble_tabs(k):
                        # gather the qb=k pieces from all cores into ctxT,
                        # build the A/B table halves, store btab half to DRAM
                        for g in range(NCORES):
                            hh, half = g // 2, g % 2
                            nc.sync.dma_start(
                                out=ctxT_sb[hh * DH:(hh + 1) * DH,
                                            half * NQ + k * QB:half * NQ + (k + 1) * QB],
                                in_=cc_out[k][g, :, :])
                        blocks = [8 * k + i for i in range(8)] + \
                                 [16 + 8 * k + i for i in range(8)]
                        for t in blocks:
                            tp = tab_ps.tile([128, 256], F32, tag="tab")
                            nc.tensor.matmul(tp[:, 0:128], ones_sb[...], bias_sb[...],
                                             start=True, stop=False)
                            nc.tensor.matmul(tp[:, 0:128], ctxT_sb[:, t * 128:(t + 1) * 128],
                                             wa_sb[...], start=False, stop=True)
                            nc.tensor.matmul(tp[:, 128:256], ctxT_sb[:, t * 128:(t + 1) * 128],
                                             wb_sb[...], start=True, stop=True)
                            nc.scalar.activation(atab[:, t * 128:(t + 1) * 128],
                                                 tp[:, 0:128], AF.Copy)
                            nc.vector.tensor_copy(btab[:, t * 128:(t + 1) * 128],
                                                  tp[:, 128:256])
                        for rb in (128 * 8 * k, 2048 + 128 * 8 * k):
                            c0 = rb // 128
                            nc.sync.dma_start(
                                out=btab_d.ap()[rb:rb + 1024, :]
                                    .rearrange("(c p) d -> p c d", p=128),
                                in_=btab[:, c0 * 128:(c0 + 8) * 128]
                                    .rearrange("p (c d) -> p c d", d=128))

                    rg = [list(range(NCORES))]
                    attention_qb(0)
                    nc.gpsimd.collective_compute(
                        "AllGather", ALU.bypass, replica_groups=rg,
                        ins=[cc_in[0].ap()], outs=[cc_out[0].ap()])
                    prep(3)
                    prep(4)
                    attention_qb(1)
                    nc.gpsimd.collective_compute(
                        "AllGather", ALU.bypass, replica_groups=rg,
                        ins=[cc_in[1].ap()], outs=[cc_out[1].ap()])
                    prep(5)
                    assemble_tabs(0)
                    assemble_tabs(1)

                # gpsimd: the dummy prep reads btab_d AFTER the stores in
                # program order, so Tile gives it RAW waits on all four
                # store DMAs; the triggers behind it then fire only once
                # the B table has landed.
                nc.gpsimd.dma_gather(
                    dgdummy[...], btab_d[...], dst_sb[:, 0:8],
                    num_idxs=128, num_idxs_reg=128, elem_size=128,
                    transpose=False, single_packet=False,
                    prepare_only=True, sem=dummy_sem, queue_num=0)
                nc.gpsimd.dma_gather(
                    dgdummy1[...], btab_d[...], dst_sb[:, 0:8],
                    num_idxs=128, num_idxs_reg=128, elem_size=128,
                    transpose=False, single_packet=False,
                    prepare_only=True, sem=dummy_sem1, queue_num=1)
                nc.gpsimd.trigger_dma(count=None, queue_num=0)  # 0,2,4 + dummy
                nc.gpsimd.trigger_dma(count=None, queue_num=1)  # 1,3,5 + dummy1
                prep(6)
                nc.gpsimd.trigger_dma(count=None, queue_num=0)
                prep(7)
                nc.gpsimd.trigger_dma(count=None, queue_num=1)

            with (
                tc.tile_pool(name="oh", bufs=3) as oh_pool,
                tc.tile_pool(name="src_ps", bufs=8, space="PSUM") as src_ps,
                tc.tile_pool(name="hseq", bufs=3) as hpool,
                tc.tile_pool(name="sig", bufs=1) as sig_pool,
            ):
                lsp = sig_pool.tile([128, C // 128], F32, tag="lsp")
                lsn = sig_pool.tile([128, C // 128], F32, tag="lsn")
                for g in range(NCHUNK):
                    sz = sizes[g]
                    oh_sb = oh_pool.tile([128, sz], BF16, tag="oh")
                    nc.sync.dma_start(out=oh_sb[...],
                                      in_=oneh[:, offs[g]:offs[g + 1]])
                    dg = dg_tiles[g]
                    ct0 = offs[g] // 128
                    for b in range(sz // 512):
                        ps = src_ps.tile([128, 512], F32, tag="sp")
                        for t in range(4):
                            gt = ct0 + b * 4 + t      # global 128-pair tile
                            I = i_of_tile[gt]          # src block index
                            nc.tensor.matmul(
                                ps[:, t * 128:(t + 1) * 128],
                                oh_sb[:, (b * 4 + t) * 128:(b * 4 + t + 1) * 128],
                                atab[:, I * 128:(I + 1) * 128],
                                start=True, stop=True)
                        hs = hpool.tile([128, 512], BF16, tag="hs")
                        nc.vector.tensor_add(
                            hs[...], ps[...],
                            dg[:, b * 4:(b + 1) * 4, :].rearrange("p b d -> p (b d)")
                        )._wait_ge(dg_sems[g], 16)
                        hr = hpool.tile([128, 4, 128], BF16, tag="hr")
                        nc.scalar.activation(
                            hr[...].rearrange("p b d -> p (b d)"), hs[...], AF.Relu)
                        ob = ct0 + b * 4
                        nc.vector.tensor_reduce(
                            lsp[:, ob:ob + 4].rearrange("p b -> p b ()"),
                            hr[:, :, 0:npos], op=ALU.add, axis=mybir.AxisListType.X)
                        nc.vector.tensor_reduce(
                            lsn[:, ob:ob + 4].rearrange("p b -> p b ()"),
                            hr[:, :, npos:H], op=ALU.add, axis=mybir.AxisListType.X)
                logit = sig_pool.tile([128, C // 128], F32, tag="logit")
                nc.vector.tensor_sub(logit[...], lsp[...], lsn[...])
                sig_sb = sig_pool.tile([128, C // 128], F32, tag="sig")
                nc.scalar.activation(sig_sb[...], logit[...], AF.Tanh,
                                     scale=0.5, bias=0.5 * b2val)
                nc.vector.tensor_scalar(sig_sb[...], sig_sb[...], 0.5, 0.5,
                                        op0=ALU.mult, op1=ALU.add)
                nc.sync.dma_start(out=preds[...], in_=sig_sb[...])
    nc.compile()
    return nc


def _prep_inputs(inputs):
    x = np.asarray(inputs["context_embeddings"], np.float32)
    ipw = np.asarray(inputs["in_proj_w"], np.float32)
    ipb = np.asarray(inputs["in_proj_b"], np.float32)
    opw = np.asarray(inputs["out_proj_w"], np.float32)
    opb = np.asarray(inputs["out_proj_b"], np.float32)
    w1 = np.asarray(inputs["w1"], np.float32)
    b1 = np.asarray(inputs["b1"], np.float32)
    w2v = np.asarray(inputs["w2"], np.float32)[0]
    rel = np.asarray(inputs["relationships"])

    bf = ml_dtypes.bfloat16
    scale = 1.0 / np.sqrt(DH)
    xT_a = np.ascontiguousarray(x.T).astype(bf)
    w1a, w1b = w1[:, :H], w1[:, H:]
    WA = w1a @ opw
    WB = w1b @ opw
    bv_full = ipb[2 * H:]
    node_bias = opw @ bv_full + opb
    bias_total = ((w1a + w1b) @ node_bias + b1).astype(np.float32)
    perm = np.argsort(w2v < 0, kind="stable")
    npos = int((w2v >= 0).sum())
    aw2 = np.abs(w2v)[perm]
    WA_t = np.ascontiguousarray((WA[perm] * aw2[:, None]).T).astype(bf)
    WB_t = np.ascontiguousarray((WB[perm] * aw2[:, None]).T).astype(bf)
    bias_perm = (bias_total[perm] * aw2).astype(np.float32)

    # --- pair rebalancing: deal each src-block's pairs round-robin to cores,
    # so every (core, block) bucket fits kb = roundup(ceil(count_b/8), 128)
    src = rel[:, 0].astype(np.int64)
    dst = rel[:, 1].astype(np.int64)
    blk = src // 128
    counts = np.bincount(blk, minlength=32)
    kb = ((np.ceil(counts / NCORES).astype(np.int64) + 127) // 128) * 128
    C = int(kb.sum())
    padc = (-C) % 512
    kb[31] += padc
    C += padc
    block_off = np.concatenate([[0], np.cumsum(kb)[:-1]]).astype(np.int64)
    i_of_tile = np.repeat(np.arange(32), kb // 128)
    assert len(i_of_tile) == C // 128

    order = np.argsort(blk, kind="stable")
    starts = np.concatenate([[0], np.cumsum(counts)[:-1]])
    bs = blk[order]
    j_within = np.arange(P) - starts[bs]
    core_of = j_within % NCORES
    rank = j_within // NCORES
    col = block_off[bs] + rank

    # chunk sizes: C/512 units split across 8 chunks (larger chunks first)
    units = C // 512
    u = [units // NCHUNK] * NCHUNK
    for i in range(units % NCHUNK):
        u[i] += 1
    sizes = [x * 512 for x in u]
    offs = np.concatenate([[0], np.cumsum(sizes)]).astype(int)

    def wrap_idx(v):
        parts = []
        for g in range(NCHUNK):
            vg = v[offs[g]:offs[g + 1]].astype(np.int16)
            parts.append(vg.reshape(sizes[g] // 16, 16).T)
        b = np.hstack(parts)
        return np.ascontiguousarray(np.tile(b, (8, 1)))

    in_maps = []
    unshard = []
    for c in range(NCORES):
        h, half = c // 2, c % 2
        qoff = half * NQ
        m = core_of == c
        cols_c = col[m]
        src_c = src[order][m]
        dst_c = dst[order][m]
        orig_c = order[m]
        unshard.append((orig_c, cols_c))

        dst_cols = np.zeros(C, np.int64)
        dst_cols[cols_c] = dst_c
        onehot = np.zeros((128, C), np.float32)
        onehot[src_c % 128, cols_c] = 1.0

        wqh = (ipw[DH * h:DH * (h + 1), :] * scale).T
        wkh = ipw[H + DH * h:H + DH * (h + 1), :].T
        wvh = ipw[2 * H + DH * h:2 * H + DH * (h + 1), :].T
        bqh = ipb[DH * h:DH * (h + 1)] * scale
        in_maps.append({
            "xT": xT_a,
            "xTq": np.ascontiguousarray(xT_a[:, qoff:qoff + NQ]),
            "wq_t": np.ascontiguousarray(np.tile(wqh, (1, RREP))).astype(bf),
            "wk_t": np.ascontiguousarray(np.tile(wkh, (1, RREP))).astype(bf),
            "wv_t": np.ascontiguousarray(wvh).astype(bf),
            "bq": np.tile(bqh, RREP).reshape(DH * RREP, 1).astype(np.float32),
            "wa_t": WA_t, "wb_t": WB_t,
            "bias_r": bias_perm.reshape(1, H).astype(bf),
            "dstw": wrap_idx(dst_cols),
            "oneh": np.ascontiguousarray(onehot.astype(bf)),
        })
    return in_maps, npos, C, sizes, i_of_tile, unshard


def kernel(**inputs):
    from concourse import bass_utils

    in_maps, npos, C, sizes, i_of_tile, unshard = _prep_inputs(inputs)
    b2val = float(np.asarray(inputs["b2"], np.float32)[0])
    nc = _build(npos, b2val, C, sizes, i_of_tile)
    res = bass_utils.run_bass_kernel_spmd(
        nc, in_maps, core_ids=list(range(NCORES)), trace=False)
    out = np.empty(P, np.float32)
    for c in range(NCORES):
        pr = res.results[c]["preds"]           # [128, C//128]
        flat = pr.T.reshape(-1)                # col-major: col = b*128 + p
        orig_c, cols_c = unshard[c]
        out[orig_c] = flat[cols_c]
    return out.astype(np.float32)


# revision 14
# speedup vs baseline: 1.5994x; 1.0279x over previous
"""Trainium2 Bass kernel for nn_DroneRelationModel (8 NeuronCores).

Strategy v3 (gather-overlap):
  - Attention sharded (head, query-half) across 8 cores; the AllGather is
    split per query-quarter (qb) so the B-table halves land earlier.
  - dst-gather desc-gen (the bottleneck: ~7-8ns/idx on GpSimd) runs from
    t~0: prepare_only dma_gathers are emitted BEFORE any write to btab_d
    (so Tile attaches no RAW dep to them), interleaved with the two
    collective_computes on the gpsimd queue.  All ~35k descriptors can sit
    untriggered because they are spread over TWO SWDGE queues with a 64KB
    descriptor ring carveout (measured: one 32KB queue caps out between
    18k and 23k pending indices and the kernel deadlocks).
  - Trigger gating: a tiny dummy prep emitted AFTER the btab stores picks
    up Tile's RAW waits on the four store DMAs; the per-queue triggers
    behind it fire only once the B table has landed.  Later chunks get
    their own triggers for drain/compute pipelining.
  - Pair bucketing compacted: pairs rebalanced across cores per src-block
    (round-robin deal), so every (core, block) bucket fits in
    kb = roundup(ceil(count_b/8), 128) columns -> C ~ 34k instead of 41k.
  - Head: src rows via one-hot matmuls vs A table (SBUF), dst rows via the
    gathers, add on DVE, relu on Scalar (ACT), sign-split reduces on DVE.
All heavy matmul inputs in bf16.
"""
import numpy as np
import ml_dtypes

N, H, HEADS, P = 4096, 128, 4, 262144
DH = 32
NCORES = 8
NQ = 2048
QB = 1024
RREP = 3
NCHUNK = 8


def _build(npos, b2val, C, sizes, i_of_tile):
    import concourse.bass as bass
    import concourse.mybir as mybir
    import concourse.tile as tile
    from concourse import bacc

    F32 = mybir.dt.float32
    BF16 = mybir.dt.bfloat16
    I16 = mybir.dt.int16
    AF = mybir.ActivationFunctionType
    ALU = mybir.AluOpType

    nc = bacc.Bacc("TRN2", target_bir_lowering=False, debug=False,
                   num_devices=NCORES, dynamic_dma_scratch_size=65536,
                   num_swdge_queues=2)

    xT = nc.dram_tensor("xT", [H, N], BF16, kind="ExternalInput")
    xTq = nc.dram_tensor("xTq", [H, NQ], BF16, kind="ExternalInput")
    wq_t = nc.dram_tensor("wq_t", [H, DH * RREP], BF16, kind="ExternalInput")
    wk_t = nc.dram_tensor("wk_t", [H, DH * RREP], BF16, kind="ExternalInput")
    wv_t = nc.dram_tensor("wv_t", [H, DH], BF16, kind="ExternalInput")
    bq = nc.dram_tensor("bq", [DH * RREP, 1], F32, kind="ExternalInput")
    wa_t = nc.dram_tensor("wa_t", [H, H], BF16, kind="ExternalInput")
    wb_t = nc.dram_tensor("wb_t", [H, H], BF16, kind="ExternalInput")
    bias_r = nc.dram_tensor("bias_r", [1, H], BF16, kind="ExternalInput")
    dstw = nc.dram_tensor("dstw", [128, C // 16], I16, kind="ExternalInput")
    oneh = nc.dram_tensor("oneh", [128, C], BF16, kind="ExternalInput")
    preds = nc.dram_tensor("preds", [128, C // 128], F32, kind="ExternalOutput")
    cc_in = [nc.dram_tensor(f"cc_in{k}", [DH, QB], BF16) for k in range(2)]
    cc_out = [nc.dram_tensor(f"cc_out{k}", [NCORES, DH, QB], BF16,
                             addr_space="Shared") for k in range(2)]
    btab_d = nc.dram_tensor("btab_d", [N, H], BF16)

    offs = np.concatenate([[0], np.cumsum(sizes)]).astype(int)

    with tile.TileContext(nc) as tc:
        with tc.tile_pool(name="const", bufs=1) as cpool:
            def cload(pool, name, dram, shape, dtype):
                t = pool.tile(shape, dtype, tag=name)
                nc.sync.dma_start(out=t[...], in_=dram[...])
                return t

            # index data first so preps can start early
            dst_sb = cload(cpool, "dst", dstw, [128, C // 16], I16)
            wq_sb = cload(cpool, "wq", wq_t, [H, DH * RREP], BF16)
            wk_sb = cload(cpool, "wk", wk_t, [H, DH * RREP], BF16)
            wv_sb = cload(cpool, "wv", wv_t, [H, DH], BF16)
            bq_sb = cload(cpool, "bq", bq, [DH * RREP, 1], F32)
            wa_sb = cload(cpool, "wa", wa_t, [H, H], BF16)
            wb_sb = cload(cpool, "wb", wb_t, [H, H], BF16)
            bias_sb = cload(cpool, "bias", bias_r, [1, H], BF16)

            # dst-gather output tiles (addresses fixed before preps run)
            dg_tiles = []
            for g in range(NCHUNK):
                dg = cpool.tile([128, sizes[g] // 128, 128], BF16, tag=f"dg{g}")
                dg_tiles.append(dg)
            dgdummy = cpool.tile([128, 1, 128], BF16, tag="dgdummy")
            dgdummy1 = cpool.tile([128, 1, 128], BF16, tag="dgdummy1")
            atab = cpool.tile([128, 32 * 128], BF16, tag="atab")

            dg_sems = [nc.alloc_semaphore(f"dgsem{g}") for g in range(NCHUNK)]
            dummy_sem = nc.alloc_semaphore("dummysem")
            dummy_sem1 = nc.alloc_semaphore("dummysem1")

            def prep(g):
                iw0, iw1 = offs[g] // 16, offs[g + 1] // 16
                nc.gpsimd.dma_gather(
                    dg_tiles[g][...], btab_d[...],
                    dst_sb[:, iw0:iw1],
                    num_idxs=sizes[g], num_idxs_reg=sizes[g], elem_size=128,
                    transpose=False, single_packet=False,
                    prepare_only=True, sem=dg_sems[g], queue_num=g % 2)

            prep(0)
            prep(1)
            prep(2)

            with tc.tile_pool(name="attn", bufs=1) as apool:
                xT_sb = cload(apool, "xT", xT, [H, N], BF16)
                xTq_sb = cload(apool, "xTq", xTq, [H, NQ], BF16)
                kT_sb = apool.tile([DH * RREP, N], BF16, tag="kT")
                qT_sb = apool.tile([DH * RREP, NQ], BF16, tag="qT")
                v_sb = apool.tile([128, 33 * 32], BF16, tag="v")
                ctxn_sb = apool.tile([DH, NQ], BF16, tag="ctxn")
                ctxT_sb = apool.tile([H, N], BF16, tag="ctxT")
                btab = apool.tile([128, 32 * 128], BF16, tag="btab")
                ones_sb = apool.tile([1, H], BF16, tag="ones")
                nc.vector.memset(ones_sb[...], 1.0)

                # ---- qkv projections ----
                with tc.tile_pool(name="kqv_ps", bufs=2, space="PSUM") as kqv_ps:
                    for i in range(4):
                        ps = kqv_ps.tile([DH * RREP, QB], F32, tag="kq")
                        for j in range(2):
                            nc.tensor.matmul(ps[:, j * 512:(j + 1) * 512], wk_sb[...],
                                             xT_sb[:, i * QB + j * 512:i * QB + (j + 1) * 512],
                                             start=True, stop=True)
                        nc.vector.tensor_copy(kT_sb[:, i * QB:(i + 1) * QB], ps[...])
                    for i in range(2):
                        ps = kqv_ps.tile([DH * RREP, QB], F32, tag="kq")
                        for j in range(2):
                            nc.tensor.matmul(ps[:, j * 512:(j + 1) * 512], wq_sb[...],
                                             xTq_sb[:, i * QB + j * 512:i * QB + (j + 1) * 512],
                                             start=True, stop=True)
                        nc.vector.tensor_scalar_add(qT_sb[:, i * QB:(i + 1) * QB],
                                                    ps[...], bq_sb[...])
                    nc.vector.memset(v_sb[...], 1.0)
                    with tc.tile_pool(name="v_ps", bufs=4, space="PSUM") as v_ps:
                        for kc in range(32):
                            ps = v_ps.tile([128, DH], F32, tag="v")
                            nc.tensor.matmul(ps[...], xT_sb[:, kc * 128:(kc + 1) * 128],
                                             wv_sb[...], start=True, stop=True)
                            nc.vector.tensor_copy(v_sb[:, kc * 33:kc * 33 + 32], ps[...])

                with (
                    tc.tile_pool(name="s_ps", bufs=2, space="PSUM") as s_ps,
                    tc.tile_pool(name="av_ps", bufs=1, space="PSUM") as av_psp,
                    tc.tile_pool(name="tab_ps", bufs=2, space="PSUM") as tab_ps,
                    tc.tile_pool(name="es", bufs=6) as es_pool,
                    tc.tile_pool(name="misc", bufs=1) as misc,
                ):
                    def attention_qb(qb):
                        av_ps = av_psp.tile([128, QB], F32, tag="av")
                        es_tiles = {}
                        for kc in range(32):
                            r = kc % RREP
                            sp = s_ps.tile([128, QB], F32, tag="s")
                            for j in range(2):
                                nc.tensor.matmul(
                                    sp[:, j * 512:(j + 1) * 512],
                                    kT_sb[r * DH:(r + 1) * DH, kc * 128:(kc + 1) * 128],
                                    qT_sb[r * DH:(r + 1) * DH,
                                          qb * QB + j * 512:qb * QB + (j + 1) * 512],
                                    start=True, stop=True, tile_position=(r * DH, 0))
                            es = es_pool.tile([128, QB], BF16, tag="es")
                            nc.scalar.activation(es[...], sp[...], AF.Exp)
                            es_tiles[kc] = es
                            if kc % 2 == 1:
                                for t, kk in ((0, kc - 1), (1, kc)):
                                    for j in range(2):
                                        nc.tensor.matmul(
                                            av_ps[t * 64:t * 64 + 33, j * 512:(j + 1) * 512],
                                            v_sb[:, kk * 33:(kk + 1) * 33],
                                            es_tiles[kk][:, j * 512:(j + 1) * 512],
                                            start=(kk < 2), stop=(kk >= 30),
                                            tile_position=(0, t * 64),
                                            skip_group_check=True)
                                es_tiles.clear()
                        craw = misc.tile([33, QB], F32, tag="craw")
                        t1c = misc.tile([33, QB], F32, tag="t1c")
                        nc.vector.tensor_copy(t1c[...], av_ps[64:97, :])
                        nc.vector.tensor_add(craw[...], av_ps[0:33, :], t1c[...])
                        r_sb = misc.tile([1, QB], F32, tag="r")
                        nc.vector.reciprocal(r_sb[...], craw[32:33, :])
                        rbf = misc.tile([1, QB], BF16, tag="rbf")
                        nc.vector.tensor_copy(rbf[...], r_sb[...])
                        # bc from av pool (not s_ps): s_ps rotation must stay
                        # free so the next qb's score matmuls start promptly
                        bc_ps = av_psp.tile([DH, QB], F32, tag="av")
                        for j in range(2):
                            nc.tensor.matmul(bc_ps[:, j * 512:(j + 1) * 512],
                                             ones_sb[:, 0:DH], rbf[:, j * 512:(j + 1) * 512],
                                             start=True, stop=True)
                        nc.vector.tensor_mul(ctxn_sb[:, qb * QB:(qb + 1) * QB],
                                             craw[0:32, :], bc_ps[...])
                        nc.sync.dma_start(out=cc_in[qb][...],
                                          in_=ctxn_sb[:, qb * QB:(qb + 1) * QB])

                    def assemble_tabs(k):
                        # gather the qb=k pieces from all cores into ctxT,
                        # build the A/B table halves, store btab half to DRAM
                        for g in range(NCORES):
                            hh, half = g // 2, g % 2
                            nc.sync.dma_start(
                                out=ctxT_sb[hh * DH:(hh + 1) * DH,
                                            half * NQ + k * QB:half * NQ + (k + 1) * QB],
                                in_=cc_out[k][g, :, :])
                        blocks = [8 * k + i for i in range(8)] + \
                                 [16 + 8 * k + i for i in range(8)]
                        for t in blocks:
                            tp = tab_ps.tile([128, 256], F32, tag="tab")
                            nc.tensor.matmul(tp[:, 0:128], ones_sb[...], bias_sb[...],
                                             start=True, stop=False)
                            nc.tensor.matmul(tp[:, 0:128], ctxT_sb[:, t * 128:(t + 1) * 128],
                                             wa_sb[...], start=False, stop=True)
                            nc.tensor.matmul(tp[:, 128:256], ctxT_sb[:, t * 128:(t + 1) * 128],
                                             wb_sb[...], start=True, stop=True)
                            nc.scalar.activation(atab[:, t * 128:(t + 1) * 128],
                                                 tp[:, 0:128], AF.Copy)
                            nc.vector.tensor_copy(btab[:, t * 128:(t + 1) * 128],
                                                  tp[:, 128:256])
                        for rb in (128 * 8 * k, 2048 + 128 * 8 * k):
                            c0 = rb // 128
                            nc.sync.dma_start(
                                out=btab_d.ap()[rb:rb + 1024, :]
                                    .rearrange("(c p) d -> p c d", p=128),
                                in_=btab[:, c0 * 128:(c0 + 8) * 128]
                                    .rearrange("p (c d) -> p c d", d=128))

                    rg = [list(range(NCORES))]
                    prep(3)
                    attention_qb(0)
                    nc.gpsimd.collective_compute(
                        "AllGather", ALU.bypass, replica_groups=rg,
                        ins=[cc_in[0].ap()], outs=[cc_out[0].ap()])
                    prep(4)
                    prep(5)
                    prep(6)
                    attention_qb(1)
                    nc.gpsimd.collective_compute(
                        "AllGather", ALU.bypass, replica_groups=rg,
                        ins=[cc_in[1].ap()], outs=[cc_out[1].ap()])
                    prep(7)
                    assemble_tabs(0)
                    assemble_tabs(1)

                # gpsimd: the dummy prep reads btab_d AFTER the stores in
                # program order, so Tile gives it RAW waits on all four
                # store DMAs; the triggers behind it then fire only once
                # the B table has landed.
                nc.gpsimd.dma_gather(
                    dgdummy[...], btab_d[...], dst_sb[:, 0:8],
                    num_idxs=128, num_idxs_reg=128, elem_size=128,
                    transpose=False, single_packet=False,
                    prepare_only=True, sem=dummy_sem, queue_num=0)
                nc.gpsimd.dma_gather(
                    dgdummy1[...], btab_d[...], dst_sb[:, 0:8],
                    num_idxs=128, num_idxs_reg=128, elem_size=128,
                    transpose=False, single_packet=False,
                    prepare_only=True, sem=dummy_sem1, queue_num=1)
                nc.gpsimd.trigger_dma(count=None, queue_num=0)  # 0,2,4,6 + dummy
                nc.gpsimd.trigger_dma(count=None, queue_num=1)  # 1,3,5,7 + dummy1

            with (
                tc.tile_pool(name="oh", bufs=3) as oh_pool,
                tc.tile_pool(name="src_ps", bufs=8, space="PSUM") as src_ps,
                tc.tile_pool(name="hseq", bufs=3) as hpool,
                tc.tile_pool(name="sig", bufs=1) as sig_pool,
            ):
                lsp = sig_pool.tile([128, C // 128], F32, tag="lsp")
                lsn = sig_pool.tile([128, C // 128], F32, tag="lsn")
                for g in range(NCHUNK):
                    sz = sizes[g]
                    oh_sb = oh_pool.tile([128, sz], BF16, tag="oh")
                    nc.sync.dma_start(out=oh_sb[...],
                                      in_=oneh[:, offs[g]:offs[g + 1]])
                    dg = dg_tiles[g]
                    ct0 = offs[g] // 128
                    for b in range(sz // 512):
                        ps = src_ps.tile([128, 512], F32, tag="sp")
                        for t in range(4):
                            gt = ct0 + b * 4 + t      # global 128-pair tile
                            I = i_of_tile[gt]          # src block index
                            nc.tensor.matmul(
                                ps[:, t * 128:(t + 1) * 128],
                                oh_sb[:, (b * 4 + t) * 128:(b * 4 + t + 1) * 128],
                                atab[:, I * 128:(I + 1) * 128],
                                start=True, stop=True)
                        hs = hpool.tile([128, 512], BF16, tag="hs")
                        nc.vector.tensor_add(
                            hs[...], ps[...],
                            dg[:, b * 4:(b + 1) * 4, :].rearrange("p b d -> p (b d)")
                        )._wait_ge(dg_sems[g], 16)
                        hr = hpool.tile([128, 4, 128], BF16, tag="hr")
                        nc.scalar.activation(
                            hr[...].rearrange("p b d -> p (b d)"), hs[...], AF.Relu)
                        ob = ct0 + b * 4
                        nc.vector.tensor_reduce(
                            lsp[:, ob:ob + 4].rearrange("p b -> p b ()"),
                            hr[:, :, 0:npos], op=ALU.add, axis=mybir.AxisListType.X)
                        nc.vector.tensor_reduce(
                            lsn[:, ob:ob + 4].rearrange("p b -> p b ()"),
                            hr[:, :, npos:H], op=ALU.add, axis=mybir.AxisListType.X)
                logit = sig_pool.tile([128, C // 128], F32, tag="logit")
                nc.vector.tensor_sub(logit[...], lsp[...], lsn[...])
                sig_sb = sig_pool.tile([128, C // 128], F32, tag="sig")
                nc.scalar.activation(sig_sb[...], logit[...], AF.Tanh,
                                     scale=0.5, bias=0.5 * b2val)
                nc.vector.tensor_scalar(sig_sb[...], sig_sb[...], 0.5, 0.5,
                                        op0=ALU.mult, op1=ALU.add)
                nc.sync.dma_start(out=preds[...], in_=sig_sb[...])
    nc.compile()
    return nc


def _prep_inputs(inputs):
    x = np.asarray(inputs["context_embeddings"], np.float32)
    ipw = np.asarray(inputs["in_proj_w"], np.float32)
    ipb = np.asarray(inputs["in_proj_b"], np.float32)
    opw = np.asarray(inputs["out_proj_w"], np.float32)
    opb = np.asarray(inputs["out_proj_b"], np.float32)
    w1 = np.asarray(inputs["w1"], np.float32)
    b1 = np.asarray(inputs["b1"], np.float32)
    w2v = np.asarray(inputs["w2"], np.float32)[0]
    rel = np.asarray(inputs["relationships"])

    bf = ml_dtypes.bfloat16
    scale = 1.0 / np.sqrt(DH)
    xT_a = np.ascontiguousarray(x.T).astype(bf)
    w1a, w1b = w1[:, :H], w1[:, H:]
    WA = w1a @ opw
    WB = w1b @ opw
    bv_full = ipb[2 * H:]
    node_bias = opw @ bv_full + opb
    bias_total = ((w1a + w1b) @ node_bias + b1).astype(np.float32)
    perm = np.argsort(w2v < 0, kind="stable")
    npos = int((w2v >= 0).sum())
    aw2 = np.abs(w2v)[perm]
    WA_t = np.ascontiguousarray((WA[perm] * aw2[:, None]).T).astype(bf)
    WB_t = np.ascontiguousarray((WB[perm] * aw2[:, None]).T).astype(bf)
    bias_perm = (bias_total[perm] * aw2).astype(np.float32)

    # --- pair rebalancing: deal each src-block's pairs round-robin to cores,
    # so every (core, block) bucket fits kb = roundup(ceil(count_b/8), 128)
    src = rel[:, 0].astype(np.int64)
    dst = rel[:, 1].astype(np.int64)
    blk = src // 128
    counts = np.bincount(blk, minlength=32)
    kb = ((np.ceil(counts / NCORES).astype(np.int64) + 127) // 128) * 128
    C = int(kb.sum())
    padc = (-C) % 512
    kb[31] += padc
    C += padc
    block_off = np.concatenate([[0], np.cumsum(kb)[:-1]]).astype(np.int64)
    i_of_tile = np.repeat(np.arange(32), kb // 128)
    assert len(i_of_tile) == C // 128

    order = np.argsort(blk, kind="stable")
    starts = np.concatenate([[0], np.cumsum(counts)[:-1]])
    bs = blk[order]
    j_within = np.arange(P) - starts[bs]
    core_of = j_within % NCORES
    rank = j_within // NCORES
    col = block_off[bs] + rank

    # chunk sizes: C/512 units split across 8 chunks (larger chunks first)
    units = C // 512
    u = [units // NCHUNK] * NCHUNK
    for i in range(units % NCHUNK):
        u[i] += 1
    sizes = [x * 512 for x in u]
    offs = np.concatenate([[0], np.cumsum(sizes)]).astype(int)

    def wrap_idx(v):
        parts = []
        for g in range(NCHUNK):
            vg = v[offs[g]:offs[g + 1]].astype(np.int16)
            parts.append(vg.reshape(sizes[g] // 16, 16).T)
        b = np.hstack(parts)
        return np.ascontiguousarray(np.tile(b, (8, 1)))

    in_maps = []
    unshard = []
    for c in range(NCORES):
        h, half = c // 2, c % 2
        qoff = half * NQ
        m = core_of == c
        cols_c = col[m]
        src_c = src[order][m]
        dst_c = dst[order][m]
        orig_c = order[m]
        unshard.append((orig_c, cols_c))

        dst_cols = np.zeros(C, np.int64)
        dst_cols[cols_c] = dst_c
        onehot = np.zeros((128, C), np.float32)
        onehot[src_c % 128, cols_c] = 1.0

        wqh = (ipw[DH * h:DH * (h + 1), :] * scale).T
        wkh = ipw[H + DH * h:H + DH * (h + 1), :].T
        wvh = ipw[2 * H + DH * h:2 * H + DH * (h + 1), :].T
        bqh = ipb[DH * h:DH * (h + 1)] * scale
        in_maps.append({
            "xT": xT_a,
            "xTq": np.ascontiguousarray(xT_a[:, qoff:qoff + NQ]),
            "wq_t": np.ascontiguousarray(np.tile(wqh, (1, RREP))).astype(bf),
            "wk_t": np.ascontiguousarray(np.tile(wkh, (1, RREP))).astype(bf),
            "wv_t": np.ascontiguousarray(wvh).astype(bf),
            "bq": np.tile(bqh, RREP).reshape(DH * RREP, 1).astype(np.float32),
            "wa_t": WA_t, "wb_t": WB_t,
            "bias_r": bias_perm.reshape(1, H).astype(bf),
            "dstw": wrap_idx(dst_cols),
            "oneh": np.ascontiguousarray(onehot.astype(bf)),
        })
    return in_maps, npos, C, sizes, i_of_tile, unshard


def kernel(**inputs):
    from concourse import bass_utils

    in_maps, npos, C, sizes, i_of_tile, unshard = _prep_inputs(inputs)
    b2val = float(np.asarray(inputs["b2"], np.float32)[0])
    nc = _build(npos, b2val, C, sizes, i_of_tile)
    res = bass_utils.run_bass_kernel_spmd(
        nc, in_maps, core_ids=list(range(NCORES)), trace=False)
    out = np.empty(P, np.float32)
    for c in range(NCORES):
        pr = res.results[c]["preds"]           # [128, C//128]
        flat = pr.T.reshape(-1)                # col-major: col = b*128 + p
        orig_c, cols_c = unshard[c]
        out[orig_c] = flat[cols_c]
    return out.astype(np.float32)
